# revision 1
# baseline (speedup 1.0000x reference)
"""DeepseekV2 MoE layer on 8 Trainium2 NeuronCores (Bass/Tile, SPMD).

Strategy (expert-parallel + TP-sharded shared expert, all in fp32r):
 - Host computes the MoE gate routing in numpy (bitwise-matches the jax
   reference: top-k margins are ~1e-4, far above ulp noise).
 - 16 experts -> 8 cores, 2 "slots" per core: slot0 holds the 8 largest
   experts (one per core), slot1 the 8 smallest.  Per-slot token capacity
   is the max gathered-token count over cores (SPMD requires uniform
   shapes); tokens are gathered/padded on host.
 - Device per core: for each slot, GEMM1 (x_gathered^T @ w_gate_up^T,
   gate/up rows interleaved in 128-row pairs) -> SiLU*mul -> GEMM2
   (w_down) -> scale rows by renormalized gate weight * 2.5 -> DMA out.
   Then the shared expert, TP-sharded over its intermediate dim (352 per
   core, zero-padded to 384).
 - All matmuls run in float32r (full PE rate at N>=256, ~1e-4 rel err).
 - Host scatter-adds per-expert outputs and sums shared partials.
"""

import numpy as np
from contextlib import ExitStack

import concourse.bacc as bacc
import concourse.tile as tile
import concourse.mybir as mybir
from concourse.bass_utils import run_bass_kernel_spmd

# problem dims (fixed by the graded problem)
T, D, I, E = 1024, 2048, 1408, 16
SI = 2 * I               # shared expert intermediate (2816)
TOP_K, N_GROUP, TOPK_GROUP = 6, 4, 2
ROUTED_SCALE = 2.5
NCORES = 8
KT = D // 128            # 16 contraction tiles
IT = I // 128            # 11 intermediate tiles (routed)
MT1 = 2 * IT             # 22 gemm1 m-tiles (gate/up interleaved)
SSLICE = SI // NCORES    # 352 shared-intermediate rows per core
SIP = 384                # padded to 3x128
SIT = SIP // 128         # 3
NT2 = D // 512           # 4 gemm2 n-tiles

f32 = mybir.dt.float32
f32r = mybir.dt.float32r
ACT_SILU = mybir.ActivationFunctionType.Silu
ACT_SIGMOID = mybir.ActivationFunctionType.Sigmoid
_SIM_SILU = False  # CoreSim lacks Silu; True emits Sigmoid + explicit mul


# ---------------------------------------------------------------- routing
def _route(x, gate_w, bias):
    """Replicates the jax reference gate in numpy f32 (decision margins are
    >=1e-4 so ulp-level differences cannot flip the top-k).

    Returns topk_idx [T,6] int, weights [T,6] f32 (renormalized, unscaled).
    """
    logits = (x @ gate_w.T).astype(np.float32)
    scores = (1.0 / (1.0 + np.exp(-logits))).astype(np.float32)
    s_choice = scores + bias.astype(np.float32)
    grp = s_choice.reshape(T, N_GROUP, E // N_GROUP)
    group_scores = np.sort(grp, axis=2)[:, :, -2:].sum(2, dtype=np.float32)
    grp_idx = np.argsort(-group_scores, axis=1, kind="stable")[:, :TOPK_GROUP]
    gmask = np.zeros((T, N_GROUP), dtype=bool)
    gmask[np.arange(T)[:, None], grp_idx] = True
    emask = np.repeat(gmask, E // N_GROUP, axis=1)
    masked = np.where(emask, s_choice, -np.inf)
    topk_idx = np.argsort(-masked, axis=1, kind="stable")[:, :TOP_K]
    w = np.take_along_axis(scores, topk_idx, axis=1)
    w = (w / w.sum(axis=1, keepdims=True)).astype(np.float32)
    return topk_idx, w


def _chunks(c):
    """Split capacity c into GEMM1 moving-dim chunks, each in [256, 512]."""
    if c <= 512:
        return [(0, c)]
    a = 16 * ((c + 31) // 32)
    return [(0, a), (a, c - a)]


def _pad16(n):
    return max(256, 16 * ((n + 15) // 16))


# ------------------------------------------------------------ host packing
def _pack_wgu(w, it_cnt):
    """w: [2*ic, D] rows (gate block then up block, ic=128*it_cnt rows each)
    -> [2*it_cnt, 128, KT, 128] with gate/up 128-row tiles interleaved,
    laid out so tile m is a [128 part, KT*128] contiguous block of
    w^T[k-tile, m-tile]."""
    ic = 128 * it_cnt
    g = w[:ic].reshape(it_cnt, 128, D)
    u = w[ic:].reshape(it_cnt, 128, D)
    inter = np.stack([g, u], axis=1).reshape(2 * it_cnt * 128, D)  # [2ic, D]
    t = inter.T.reshape(KT, 128, 2 * it_cnt, 128).transpose(2, 1, 0, 3)
    return np.ascontiguousarray(t, dtype=np.float32)


def _pack_wd(wdT, it_cnt):
    """wdT: [128*it_cnt, D] (= w_down^T, zero-padded rows allowed)
    -> [NT2, 128, it_cnt, 512]."""
    t = wdT.reshape(it_cnt, 128, NT2, 512).transpose(2, 1, 0, 3)
    return np.ascontiguousarray(t, dtype=np.float32)


def _pack_xT(xs, cap):
    """xs: [n, D] token rows -> [128, KT, cap] (x^T k-tiles, zero padded)."""
    out = np.zeros((128, KT, cap), dtype=np.float32)
    n = xs.shape[0]
    out[:, :, :n] = xs.T.reshape(KT, 128, n).transpose(1, 0, 2)
    return out


# ------------------------------------------------------------ device build
def _build(C0, C1):
    nc = bacc.Bacc("TRN2", target_bir_lowering=False, debug=False,
                   num_devices=NCORES)
    ch0, ch1 = _chunks(C0), _chunks(C1)
    mt0 = [(r, min(128, C0 - r)) for r in range(0, C0, 128)]
    mt1 = [(r, min(128, C1 - r)) for r in range(0, C1, 128)]
    mts = [(r, 128) for r in range(0, T, 128)]

    xg0_d = nc.dram_tensor("xg0", [128, KT, C0], f32r, kind="ExternalInput")
    xg1_d = nc.dram_tensor("xg1", [128, KT, C1], f32r, kind="ExternalInput")
    xt_d = nc.dram_tensor("xt", [128, KT, T], f32r, kind="ExternalInput")
    wgu0_d = nc.dram_tensor("wgu0", [MT1, 128, KT, 128], f32r, kind="ExternalInput")
    wgu1_d = nc.dram_tensor("wgu1", [MT1, 128, KT, 128], f32r, kind="ExternalInput")
    wsgu_d = nc.dram_tensor("wsgu", [2 * SIT, 128, KT, 128], f32r, kind="ExternalInput")
    wd0_d = nc.dram_tensor("wd0", [NT2, 128, IT, 512], f32r, kind="ExternalInput")
    wd1_d = nc.dram_tensor("wd1", [NT2, 128, IT, 512], f32r, kind="ExternalInput")
    wsd_d = nc.dram_tensor("wsd", [NT2, 128, SIT, 512], f32r, kind="ExternalInput")
    cw0_d = nc.dram_tensor("cw0", [len(mt0), 128], f32, kind="ExternalInput")
    cw1_d = nc.dram_tensor("cw1", [len(mt1), 128], f32, kind="ExternalInput")
    yr0_d = nc.dram_tensor("yr0", [C0, D], f32, kind="ExternalOutput")
    yr1_d = nc.dram_tensor("yr1", [C1, D], f32, kind="ExternalOutput")
    ys_d = nc.dram_tensor("ys", [T, D], f32, kind="ExternalOutput")

    with tile.TileContext(nc) as tc, ExitStack() as ctx:
        sb = ctx.enter_context(tc.tile_pool(name="sb", bufs=1))
        ps = ctx.enter_context(tc.tile_pool(name="ps", bufs=1, space="PSUM"))

        def ffn(xg_d, cap, chunks, it_cnt, wgu_d, wd_d, cw_d, mtl, out_d):
            # load x^T (gathered) per k-tile so matmuls can chase the DMA.
            # The shared-expert x^T is split in two: the second half lives in
            # its own tag so its DMA prefetches from kernel start instead of
            # stalling the shared phase behind the slot-1 buffer release.
            if cap == T:
                h = KT // 2
                xa = sb.tile([128, h, cap], f32r, tag="xbuf", bufs=1, name="xa")
                xb = sb.tile([128, KT - h, cap], f32r, tag="xtb", bufs=1, name="xb")
                for k in range(KT):
                    dst = xa[:, k, :] if k < h else xb[:, k - h, :]
                    nc.gpsimd.dma_start(dst, xg_d.ap()[:, k, :])
                xg_at = lambda k: xa[:, k, :] if k < h else xb[:, k - h, :]
            else:
                xg = sb.tile([128, KT, cap], f32r, tag="xbuf", bufs=1, name="xg")
                for k in range(KT):
                    nc.gpsimd.dma_start(xg[:, k, :], xg_d.ap()[:, k, :])
                xg_at = lambda k: xg[:, k, :]
            cw = None
            if cw_d is not None:
                cw = sb.tile([128, len(mtl)], f32, tag="cw", bufs=2, name="cw")
                for m, _ in enumerate(mtl):
                    nc.gpsimd.dma_start(cw[:, m:m + 1], cw_d.ap()[m].unsqueeze(1))
            # GEMM1 + silu*mul -> at (A^T, [i, tokens], f32r)
            at = sb.tile([128, it_cnt, cap], f32r, tag="at", bufs=1, name="at")
            for t in range(it_cnt):
                pair = []
                for par in (0, 1):
                    wgu = sb.tile([128, KT, 128], f32r, tag="wgu", bufs=3, name="wgu")
                    nc.sync.dma_start(wgu[:], wgu_d.ap()[2 * t + par])
                    row = []
                    for off, n in chunks:
                        p = ps.tile([128, n], f32, tag=f"ps{par}", bufs=3, name=f"ps{par}")
                        for k in range(KT):
                            nc.tensor.matmul(p[:], wgu[:, k, :], xg_at(k)[:, off:off + n],
                                             start=(k == 0), stop=(k == KT - 1))
                        row.append(p)
                    pair.append(row)
                for ci, (off, n) in enumerate(chunks):
                    tmp = sb.tile([128, n], f32, tag="tmp", bufs=3, name="tmp")
                    if _SIM_SILU:
                        nc.scalar.activation(tmp[:], pair[0][ci][:], ACT_SIGMOID)
                        nc.vector.tensor_mul(tmp[:], tmp[:], pair[0][ci][:])
                    else:
                        nc.scalar.activation(tmp[:], pair[0][ci][:], ACT_SILU)
                    nc.vector.tensor_mul(at[:, t, off:off + n], tmp[:], pair[1][ci][:])
            # GEMM2 -> scale -> out
            for nt in range(NT2):
                wd = sb.tile([128, it_cnt, 512], f32r, tag="wd", bufs=2, name="wd")
                nc.sync.dma_start(wd[:], wd_d.ap()[nt])
                for mi, (r0, p_) in enumerate(mtl):
                    yp = ps.tile([128, 512], f32, tag="psy", bufs=2, name="yp")
                    for k in range(it_cnt):
                        nc.tensor.matmul(yp[:p_, :], at[:, k, r0:r0 + p_], wd[:, k, :],
                                         start=(k == 0), stop=(k == it_cnt - 1))
                    ysb = sb.tile([128, 512], f32, tag="ysb", bufs=3, name="ysb")
                    if cw is not None:
                        nc.vector.tensor_scalar_mul(ysb[:p_, :], yp[:p_, :], cw[:p_, mi:mi + 1])
                    else:
                        nc.vector.tensor_copy(ysb[:p_, :], yp[:p_, :])
                    nc.scalar.dma_start(out_d.ap()[r0:r0 + p_, nt * 512:(nt + 1) * 512],
                                        ysb[:p_, :])

        ffn(xg0_d, C0, ch0, IT, wgu0_d, wd0_d, cw0_d, mt0, yr0_d)
        ffn(xg1_d, C1, ch1, IT, wgu1_d, wd1_d, cw1_d, mt1, yr1_d)
        ffn(xt_d, T, [(0, 512), (512, 512)], SIT, wsgu_d, wsd_d, None, mts, ys_d)

    nc.compile()
    return nc


# ----------------------------------------------------------------- kernel
def kernel(x, gate_w, bias, w_gate_up, w_down, shared_w_gate_up,
           shared_w_down, _trace=False):
    x = np.ascontiguousarray(x, dtype=np.float32)
    topk_idx, w = _route(x, gate_w, bias)
    cw_full = w.astype(np.float32) * np.float32(ROUTED_SCALE)

    # expert -> token list + weight list
    toks, wts, counts = [], [], np.zeros(E, dtype=np.int64)
    for e in range(E):
        tsel, ksel = np.where(topk_idx == e)
        toks.append(tsel)
        wts.append(cw_full[tsel, ksel])
        counts[e] = len(tsel)

    order = np.argsort(-counts, kind="stable")
    slot0, slot1 = order[:NCORES], order[NCORES:]
    C0 = _pad16(int(counts[slot0].max()))
    C1 = _pad16(int(counts[slot1].max()))
    n_mt0 = (C0 + 127) // 128
    n_mt1 = (C1 + 127) // 128

    in_maps = []
    for c in range(NCORES):
        m = {}
        for s, (eid, cap, n_mt) in enumerate(
                [(int(slot0[c]), C0, n_mt0), (int(slot1[c]), C1, n_mt1)]):
            m[f"xg{s}"] = _pack_xT(x[toks[eid]], cap)
            m[f"wgu{s}"] = _pack_wgu(w_gate_up[eid], IT)
            m[f"wd{s}"] = _pack_wd(np.ascontiguousarray(w_down[eid].T), IT)
            cw = np.zeros(n_mt * 128, dtype=np.float32)
            cw[:counts[eid]] = wts[eid]
            m[f"cw{s}"] = cw.reshape(n_mt, 128)
        # shared expert slice (rows [352c, 352c+352), zero-padded to 384)
        gsl = np.zeros((2 * SIP, D), dtype=np.float32)
        gsl[:SSLICE] = shared_w_gate_up[SSLICE * c: SSLICE * (c + 1)]
        gsl[SIP:SIP + SSLICE] = shared_w_gate_up[SI + SSLICE * c: SI + SSLICE * (c + 1)]
        m["wsgu"] = _pack_wgu(gsl, SIT)
        sdT = np.zeros((SIP, D), dtype=np.float32)
        sdT[:SSLICE] = shared_w_down[:, SSLICE * c: SSLICE * (c + 1)].T
        m["wsd"] = _pack_wd(sdT, SIT)
        m["xt"] = _pack_xT(x, T)
        in_maps.append(m)

    nc = _build(C0, C1)
    kw = {}
    if _trace:
        kw = dict(trace=True, trace_cores=list(range(NCORES)))
    res = run_bass_kernel_spmd(nc, in_maps, core_ids=list(range(NCORES)), **kw)

    y = np.zeros((T, D), dtype=np.float32)
    for c in range(NCORES):
        y += res.results[c]["ys"]
    for c in range(NCORES):
        for s, eid in enumerate((int(slot0[c]), int(slot1[c]))):
            n = int(counts[eid])
            if n:
                y[toks[eid]] += res.results[c][f"yr{s}"][:n]
    if _trace:
        return y, res
    return y



# revision 2
# speedup vs baseline: 1.2843x; 1.2843x over previous
"""DeepseekV2 MoE layer on 8 Trainium2 NeuronCores (Bass/Tile, SPMD).

Strategy (expert-parallel + TP-sharded shared expert, bf16 matmuls):
 - Host computes the MoE gate routing in numpy (bitwise-matches the jax
   reference: top-k margins are ~1e-4, far above ulp noise).
 - 16 experts -> 8 cores, 2 "slots" per core: slot0 holds the 8 largest
   experts (one per core), slot1 the 8 smallest.  Per-slot token capacity
   is the max gathered-token count over cores (SPMD requires uniform
   shapes); tokens are gathered/padded on host.
 - Device per core: for each slot, GEMM1 (x_gathered^T @ w_gate_up^T,
   gate/up rows interleaved in 128-row pairs) -> SiLU*mul -> GEMM2
   (w_down) -> scale rows by renormalized gate weight * 2.5 -> DMA out.
   Then the shared expert, TP-sharded over its intermediate dim (352 per
   core, zero-padded to 384).
 - All matmuls run in bf16 (fp32 PSUM accumulate): halves HBM traffic
   and enables fast-weight-load; rel err ~5e-3 vs the 2e-2 gate.
 - Host scatter-adds per-expert outputs and sums shared partials in f32.
"""

import numpy as np
import ml_dtypes
from contextlib import ExitStack

import concourse.bacc as bacc
import concourse.tile as tile
import concourse.mybir as mybir
from concourse.bass_utils import run_bass_kernel_spmd

# problem dims (fixed by the graded problem)
T, D, I, E = 1024, 2048, 1408, 16
SI = 2 * I               # shared expert intermediate (2816)
TOP_K, N_GROUP, TOPK_GROUP = 6, 4, 2
ROUTED_SCALE = 2.5
NCORES = 8
KT = D // 128            # 16 contraction tiles
IT = I // 128            # 11 intermediate tiles (routed)
MT1 = 2 * IT             # 22 gemm1 m-tiles (gate/up interleaved)
SSLICE = SI // NCORES    # 352 shared-intermediate rows per core
SIP = 384                # padded to 3x128
SIT = SIP // 128         # 3
NT2 = D // 512           # 4 gemm2 n-tiles

f32 = mybir.dt.float32
bf16 = mybir.dt.bfloat16
np_bf16 = ml_dtypes.bfloat16
ACT_SILU = mybir.ActivationFunctionType.Silu
ACT_SIGMOID = mybir.ActivationFunctionType.Sigmoid
_SIM_SILU = False  # CoreSim lacks Silu; True emits Sigmoid + explicit mul


# ---------------------------------------------------------------- routing
def _route(x, gate_w, bias):
    """Replicates the jax reference gate in numpy f32 (decision margins are
    >=1e-4 so ulp-level differences cannot flip the top-k).

    Returns topk_idx [T,6] int, weights [T,6] f32 (renormalized, unscaled).
    """
    logits = (x @ gate_w.T).astype(np.float32)
    scores = (1.0 / (1.0 + np.exp(-logits))).astype(np.float32)
    s_choice = scores + bias.astype(np.float32)
    grp = s_choice.reshape(T, N_GROUP, E // N_GROUP)
    group_scores = np.sort(grp, axis=2)[:, :, -2:].sum(2, dtype=np.float32)
    grp_idx = np.argsort(-group_scores, axis=1, kind="stable")[:, :TOPK_GROUP]
    gmask = np.zeros((T, N_GROUP), dtype=bool)
    gmask[np.arange(T)[:, None], grp_idx] = True
    emask = np.repeat(gmask, E // N_GROUP, axis=1)
    masked = np.where(emask, s_choice, -np.inf)
    topk_idx = np.argsort(-masked, axis=1, kind="stable")[:, :TOP_K]
    w = np.take_along_axis(scores, topk_idx, axis=1)
    w = (w / w.sum(axis=1, keepdims=True)).astype(np.float32)
    return topk_idx, w


def _chunks(c):
    """Split capacity c into GEMM1 moving-dim chunks, each <= 512."""
    if c <= 512:
        return [(0, c)]
    a = 16 * ((c + 31) // 32)
    return [(0, a), (a, c - a)]


def _pad16(n):
    return max(256, 16 * ((n + 15) // 16))


# ------------------------------------------------------------ host packing
def _pack_wgu(w, it_cnt):
    """w: [2*ic, D] rows (gate block then up block, ic=128*it_cnt rows each)
    -> [2*it_cnt, 128, KT, 128] bf16 with gate/up 128-row tiles interleaved,
    laid out so tile m is a [128 part, KT*128] contiguous block of
    w^T[k-tile, m-tile]."""
    ic = 128 * it_cnt
    g = w[:ic].reshape(it_cnt, 128, D)
    u = w[ic:].reshape(it_cnt, 128, D)
    inter = np.stack([g, u], axis=1).reshape(2 * it_cnt * 128, D)  # [2ic, D]
    t = inter.T.reshape(KT, 128, 2 * it_cnt, 128).transpose(2, 1, 0, 3)
    return np.ascontiguousarray(t).astype(np_bf16)


def _pack_wd(wdT, it_cnt):
    """wdT: [128*it_cnt, D] (= w_down^T, zero-padded rows allowed)
    -> [NT2, 128, it_cnt, 512] bf16."""
    t = wdT.reshape(it_cnt, 128, NT2, 512).transpose(2, 1, 0, 3)
    return np.ascontiguousarray(t).astype(np_bf16)


def _pack_xT(xs, cap):
    """xs: [n, D] token rows -> [128, KT, cap] bf16 (x^T k-tiles, padded)."""
    out = np.zeros((128, KT, cap), dtype=np_bf16)
    n = xs.shape[0]
    out[:, :, :n] = xs.T.reshape(KT, 128, n).transpose(1, 0, 2).astype(np_bf16)
    return out


# ------------------------------------------------------------ device build
def _build(C0, C1):
    nc = bacc.Bacc("TRN2", target_bir_lowering=False, debug=False,
                   num_devices=NCORES)
    ch0, ch1 = _chunks(C0), _chunks(C1)
    mt0 = [(r, min(128, C0 - r)) for r in range(0, C0, 128)]
    mt1 = [(r, min(128, C1 - r)) for r in range(0, C1, 128)]
    mts = [(r, 128) for r in range(0, T, 128)]

    xg0_d = nc.dram_tensor("xg0", [128, KT, C0], bf16, kind="ExternalInput")
    xg1_d = nc.dram_tensor("xg1", [128, KT, C1], bf16, kind="ExternalInput")
    xt_d = nc.dram_tensor("xt", [128, KT, T], bf16, kind="ExternalInput")
    wgu0_d = nc.dram_tensor("wgu0", [MT1, 128, KT, 128], bf16, kind="ExternalInput")
    wgu1_d = nc.dram_tensor("wgu1", [MT1, 128, KT, 128], bf16, kind="ExternalInput")
    wsgu_d = nc.dram_tensor("wsgu", [2 * SIT, 128, KT, 128], bf16, kind="ExternalInput")
    wd0_d = nc.dram_tensor("wd0", [NT2, 128, IT, 512], bf16, kind="ExternalInput")
    wd1_d = nc.dram_tensor("wd1", [NT2, 128, IT, 512], bf16, kind="ExternalInput")
    wsd_d = nc.dram_tensor("wsd", [NT2, 128, SIT, 512], bf16, kind="ExternalInput")
    cw0_d = nc.dram_tensor("cw0", [128, len(mt0)], f32, kind="ExternalInput")
    cw1_d = nc.dram_tensor("cw1", [128, len(mt1)], f32, kind="ExternalInput")
    yr0_d = nc.dram_tensor("yr0", [C0, D], bf16, kind="ExternalOutput")
    yr1_d = nc.dram_tensor("yr1", [C1, D], bf16, kind="ExternalOutput")
    ys_d = nc.dram_tensor("ys", [T, D], bf16, kind="ExternalOutput")

    with tile.TileContext(nc) as tc, ExitStack() as ctx:
        sb = ctx.enter_context(tc.tile_pool(name="sb", bufs=1))
        ps = ctx.enter_context(tc.tile_pool(name="ps", bufs=1, space="PSUM"))

        def ffn(xg_d, cap, chunks, it_cnt, wgu_d, wd_d, cw_d, mtl, out_d):
            # load x^T (gathered) per k-tile so matmuls can chase the DMA.
            # The shared-expert x^T is split in two: the second half lives in
            # its own tag so its DMA prefetches from kernel start instead of
            # stalling the shared phase behind the slot-1 buffer release.
            if cap == T:
                h = KT // 2
                xa = sb.tile([128, h, cap], bf16, tag="xbuf", bufs=1, name="xa")
                xb = sb.tile([128, KT - h, cap], bf16, tag="xtb", bufs=1, name="xb")
                for k in range(KT):
                    dst = xa[:, k, :] if k < h else xb[:, k - h, :]
                    nc.gpsimd.dma_start(dst, xg_d.ap()[:, k, :])
                xg_at = lambda k: xa[:, k, :] if k < h else xb[:, k - h, :]
            else:
                xg = sb.tile([128, KT, cap], bf16, tag="xbuf", bufs=1, name="xg")
                for k in range(KT):
                    nc.gpsimd.dma_start(xg[:, k, :], xg_d.ap()[:, k, :])
                xg_at = lambda k: xg[:, k, :]
            cw = None
            if cw_d is not None:
                cw = sb.tile([128, len(mtl)], f32, tag="cw", bufs=2, name="cw")
                nc.gpsimd.dma_start(cw[:], cw_d.ap())
            # GEMM1 + silu*mul -> at (A^T, [i, tokens], bf16)
            at = sb.tile([128, it_cnt, cap], bf16, tag="at", bufs=1, name="at")
            for t in range(it_cnt):
                pair = []
                for par in (0, 1):
                    wgu = sb.tile([128, KT, 128], bf16, tag="wgu", bufs=3, name="wgu")
                    nc.sync.dma_start(wgu[:], wgu_d.ap()[2 * t + par])
                    row = []
                    for off, n in chunks:
                        p = ps.tile([128, n], f32, tag=f"ps{par}", bufs=3, name=f"ps{par}")
                        for k in range(KT):
                            nc.tensor.matmul(p[:], wgu[:, k, :], xg_at(k)[:, off:off + n],
                                             start=(k == 0), stop=(k == KT - 1))
                        row.append(p)
                    pair.append(row)
                for ci, (off, n) in enumerate(chunks):
                    tmp = sb.tile([128, n], bf16, tag="tmp", bufs=3, name="tmp")
                    if _SIM_SILU:
                        nc.scalar.activation(tmp[:], pair[0][ci][:], ACT_SIGMOID)
                        nc.vector.tensor_mul(tmp[:], tmp[:], pair[0][ci][:])
                    else:
                        nc.scalar.activation(tmp[:], pair[0][ci][:], ACT_SILU)
                    nc.vector.tensor_mul(at[:, t, off:off + n], tmp[:], pair[1][ci][:])
            # GEMM2 -> scale -> out
            for nt in range(NT2):
                wd = sb.tile([128, it_cnt, 512], bf16, tag="wd", bufs=2, name="wd")
                nc.sync.dma_start(wd[:], wd_d.ap()[nt])
                for mi, (r0, p_) in enumerate(mtl):
                    yp = ps.tile([128, 512], f32, tag="psy", bufs=2, name="yp")
                    for k in range(it_cnt):
                        nc.tensor.matmul(yp[:p_, :], at[:, k, r0:r0 + p_], wd[:, k, :],
                                         start=(k == 0), stop=(k == it_cnt - 1))
                    ysb = sb.tile([128, 512], bf16, tag="ysb", bufs=3, name="ysb")
                    if cw is not None:
                        nc.vector.tensor_scalar_mul(ysb[:p_, :], yp[:p_, :], cw[:p_, mi:mi + 1])
                    else:
                        nc.vector.tensor_copy(ysb[:p_, :], yp[:p_, :])
                    nc.scalar.dma_start(out_d.ap()[r0:r0 + p_, nt * 512:(nt + 1) * 512],
                                        ysb[:p_, :])

        ffn(xg0_d, C0, ch0, IT, wgu0_d, wd0_d, cw0_d, mt0, yr0_d)
        ffn(xg1_d, C1, ch1, IT, wgu1_d, wd1_d, cw1_d, mt1, yr1_d)
        ffn(xt_d, T, [(0, 512), (512, 512)], SIT, wsgu_d, wsd_d, None, mts, ys_d)

    nc.compile()
    return nc


# ----------------------------------------------------------------- kernel
def kernel(x, gate_w, bias, w_gate_up, w_down, shared_w_gate_up,
           shared_w_down, _trace=False):
    x = np.ascontiguousarray(x, dtype=np.float32)
    topk_idx, w = _route(x, gate_w, bias)
    cw_full = w.astype(np.float32) * np.float32(ROUTED_SCALE)

    # expert -> token list + weight list
    toks, wts, counts = [], [], np.zeros(E, dtype=np.int64)
    for e in range(E):
        tsel, ksel = np.where(topk_idx == e)
        toks.append(tsel)
        wts.append(cw_full[tsel, ksel])
        counts[e] = len(tsel)

    order = np.argsort(-counts, kind="stable")
    slot0, slot1 = order[:NCORES], order[NCORES:]
    C0 = _pad16(int(counts[slot0].max()))
    C1 = _pad16(int(counts[slot1].max()))
    n_mt0 = (C0 + 127) // 128
    n_mt1 = (C1 + 127) // 128

    in_maps = []
    for c in range(NCORES):
        m = {}
        for s, (eid, cap, n_mt) in enumerate(
                [(int(slot0[c]), C0, n_mt0), (int(slot1[c]), C1, n_mt1)]):
            m[f"xg{s}"] = _pack_xT(x[toks[eid]], cap)
            m[f"wgu{s}"] = _pack_wgu(w_gate_up[eid], IT)
            m[f"wd{s}"] = _pack_wd(np.ascontiguousarray(w_down[eid].T), IT)
            cw = np.zeros(n_mt * 128, dtype=np.float32)
            cw[:counts[eid]] = wts[eid]
            m[f"cw{s}"] = np.ascontiguousarray(cw.reshape(n_mt, 128).T)
        # shared expert slice (rows [352c, 352c+352), zero-padded to 384)
        gsl = np.zeros((2 * SIP, D), dtype=np.float32)
        gsl[:SSLICE] = shared_w_gate_up[SSLICE * c: SSLICE * (c + 1)]
        gsl[SIP:SIP + SSLICE] = shared_w_gate_up[SI + SSLICE * c: SI + SSLICE * (c + 1)]
        m["wsgu"] = _pack_wgu(gsl, SIT)
        sdT = np.zeros((SIP, D), dtype=np.float32)
        sdT[:SSLICE] = shared_w_down[:, SSLICE * c: SSLICE * (c + 1)].T
        m["wsd"] = _pack_wd(sdT, SIT)
        m["xt"] = _pack_xT(x, T)
        in_maps.append(m)

    nc = _build(C0, C1)
    kw = {}
    if _trace:
        kw = dict(trace=True, trace_cores=list(range(NCORES)))
    res = run_bass_kernel_spmd(nc, in_maps, core_ids=list(range(NCORES)), **kw)

    y = np.zeros((T, D), dtype=np.float32)
    for c in range(NCORES):
        y += res.results[c]["ys"].astype(np.float32)
    for c in range(NCORES):
        for s, eid in enumerate((int(slot0[c]), int(slot1[c]))):
            n = int(counts[eid])
            if n:
                y[toks[eid]] += res.results[c][f"yr{s}"][:n].astype(np.float32)
    if _trace:
        return y, res
    return y


# revision 3
# speedup vs baseline: 1.3338x; 1.0386x over previous
"""DeepseekV2 MoE layer on 8 Trainium2 NeuronCores (Bass/Tile, SPMD).

Strategy (expert-parallel with intermediate-dim pair-split, bf16 matmuls):
 - Host computes the MoE gate routing in numpy (bitwise-matches the jax
   reference: top-k margins are ~1e-4, far above ulp noise).
 - 16 experts, rank-sorted by token count, are dealt into 4 groups of 4
   (group p = ranks {p, 4+p, 8+p, 12+p}).  Group p is owned by the core
   pair (2p, 2p+1): both cores process the SAME gathered tokens of all 4
   experts, but each core only computes HALF of every expert's
   intermediate dim (704 rows, zero-padded to 768 = 6 tile-pairs).  The
   two half outputs are partial sums; the host adds them.  This beats
   whole-expert placement because per-slot capacity is the max count at
   ranks {0,4,8,12} (688+432+352+272) instead of ranks {0,8} (688+352)
   at double width.
 - Device per core: for each of 4 slots, GEMM1 (x_gathered^T @ half
   w_gate_up^T, gate/up rows interleaved in 128-row pairs) -> SiLU*mul
   -> GEMM2 (half w_down) -> scale rows by renormalized gate weight *
   2.5 -> DMA out.  Then the shared expert, TP-sharded over its
   intermediate dim (352 per core, zero-padded to 384).
 - All matmuls run in bf16 (fp32 PSUM accumulate): halves HBM traffic
   and enables fast-weight-load; rel err ~5e-3 vs the 2e-2 gate.
 - Host scatter-adds per-expert partial outputs and shared partials, f32.
"""

import numpy as np
import ml_dtypes
from contextlib import ExitStack

import concourse.bacc as bacc
import concourse.tile as tile
import concourse.mybir as mybir
from concourse.bass_utils import run_bass_kernel_spmd

# problem dims (fixed by the graded problem)
T, D, I, E = 1024, 2048, 1408, 16
SI = 2 * I               # shared expert intermediate (2816)
TOP_K, N_GROUP, TOPK_GROUP = 6, 4, 2
ROUTED_SCALE = 2.5
NCORES = 8
KT = D // 128            # 16 contraction tiles
IH = I // 2              # 704 intermediate rows per half
IHP = 768                # padded to 6x128
HIT = IHP // 128         # 6 intermediate tiles per half
SSLICE = SI // NCORES    # 352 shared-intermediate rows per core
SIP = 384                # padded to 3x128
SIT = SIP // 128         # 3
NT2 = D // 512           # 4 gemm2 n-tiles
NSLOT = 4                # expert slots per core

f32 = mybir.dt.float32
bf16 = mybir.dt.bfloat16
np_bf16 = ml_dtypes.bfloat16
ACT_SILU = mybir.ActivationFunctionType.Silu
ACT_SIGMOID = mybir.ActivationFunctionType.Sigmoid
_SIM_SILU = False  # CoreSim lacks Silu; True emits Sigmoid + explicit mul


# ---------------------------------------------------------------- routing
def _route(x, gate_w, bias):
    """Replicates the jax reference gate in numpy f32 (decision margins are
    >=1e-4 so ulp-level differences cannot flip the top-k).

    Returns topk_idx [T,6] int, weights [T,6] f32 (renormalized, unscaled).
    """
    logits = (x @ gate_w.T).astype(np.float32)
    scores = (1.0 / (1.0 + np.exp(-logits))).astype(np.float32)
    s_choice = scores + bias.astype(np.float32)
    grp = s_choice.reshape(T, N_GROUP, E // N_GROUP)
    group_scores = np.sort(grp, axis=2)[:, :, -2:].sum(2, dtype=np.float32)
    grp_idx = np.argsort(-group_scores, axis=1, kind="stable")[:, :TOPK_GROUP]
    gmask = np.zeros((T, N_GROUP), dtype=bool)
    gmask[np.arange(T)[:, None], grp_idx] = True
    emask = np.repeat(gmask, E // N_GROUP, axis=1)
    masked = np.where(emask, s_choice, -np.inf)
    topk_idx = np.argsort(-masked, axis=1, kind="stable")[:, :TOP_K]
    w = np.take_along_axis(scores, topk_idx, axis=1)
    w = (w / w.sum(axis=1, keepdims=True)).astype(np.float32)
    return topk_idx, w


def _chunks(c):
    """Split capacity c into GEMM1 moving-dim chunks, each <= 512."""
    if c <= 512:
        return [(0, c)]
    a = 16 * ((c + 31) // 32)
    return [(0, a), (a, c - a)]


def _pad16(n):
    return max(128, 16 * ((n + 15) // 16))


# ------------------------------------------------------------ host packing
def _pack_wgu(w, it_cnt):
    """w: [2*ic, D] rows (gate block then up block, ic=128*it_cnt rows each)
    -> [2*it_cnt, 128, KT, 128] bf16 with gate/up 128-row tiles interleaved,
    laid out so tile m is a [128 part, KT*128] contiguous block of
    w^T[k-tile, m-tile]."""
    ic = 128 * it_cnt
    g = w[:ic].reshape(it_cnt, 128, D)
    u = w[ic:].reshape(it_cnt, 128, D)
    inter = np.stack([g, u], axis=1).reshape(2 * it_cnt * 128, D)  # [2ic, D]
    t = inter.T.reshape(KT, 128, 2 * it_cnt, 128).transpose(2, 1, 0, 3)
    return np.ascontiguousarray(t).astype(np_bf16)


def _pack_wd(wdT, it_cnt):
    """wdT: [128*it_cnt, D] (= w_down^T, zero-padded rows allowed)
    -> [NT2, 128, it_cnt, 512] bf16."""
    t = wdT.reshape(it_cnt, 128, NT2, 512).transpose(2, 1, 0, 3)
    return np.ascontiguousarray(t).astype(np_bf16)


def _pack_xT(xs, cap):
    """xs: [n, D] token rows -> [128, KT, cap] bf16 (x^T k-tiles, padded)."""
    out = np.zeros((128, KT, cap), dtype=np_bf16)
    n = xs.shape[0]
    out[:, :, :n] = xs.T.reshape(KT, 128, n).transpose(1, 0, 2).astype(np_bf16)
    return out


# ------------------------------------------------------------ device build
def _build(caps):
    nc = bacc.Bacc("TRN2", target_bir_lowering=False, debug=False,
                   num_devices=NCORES)
    slot_mtl = [[(r, min(128, c - r)) for r in range(0, c, 128)] for c in caps]
    mts = [(r, 128) for r in range(0, T, 128)]

    xg_d = [nc.dram_tensor(f"xg{s}", [128, KT, caps[s]], bf16,
                           kind="ExternalInput") for s in range(NSLOT)]
    wgu_d = [nc.dram_tensor(f"wgu{s}", [2 * HIT, 128, KT, 128], bf16,
                            kind="ExternalInput") for s in range(NSLOT)]
    wd_d = [nc.dram_tensor(f"wd{s}", [NT2, 128, HIT, 512], bf16,
                           kind="ExternalInput") for s in range(NSLOT)]
    cw_d = [nc.dram_tensor(f"cw{s}", [128, len(slot_mtl[s])], f32,
                           kind="ExternalInput") for s in range(NSLOT)]
    yr_d = [nc.dram_tensor(f"yr{s}", [caps[s], D], bf16,
                           kind="ExternalOutput") for s in range(NSLOT)]
    xt_d = nc.dram_tensor("xt", [128, KT, T], bf16, kind="ExternalInput")
    wsgu_d = nc.dram_tensor("wsgu", [2 * SIT, 128, KT, 128], bf16, kind="ExternalInput")
    wsd_d = nc.dram_tensor("wsd", [NT2, 128, SIT, 512], bf16, kind="ExternalInput")
    ys_d = nc.dram_tensor("ys", [T, D], bf16, kind="ExternalOutput")

    with tile.TileContext(nc) as tc, ExitStack() as ctx:
        sb = ctx.enter_context(tc.tile_pool(name="sb", bufs=1))
        ps = ctx.enter_context(tc.tile_pool(name="ps", bufs=1, space="PSUM"))

        def ffn(xgd, cap, chunks, it_cnt, wgud, wdd, cwd, mtl, out_d):
            # load x^T (gathered) per k-tile so matmuls can chase the DMA.
            # The shared-expert x^T lives in its own tags so its DMA
            # prefetches from kernel start instead of competing with the
            # routed slots' x buffers.
            if cap == T:
                h = KT // 2
                xa = sb.tile([128, h, cap], bf16, tag="xta", bufs=1, name="xa")
                xb = sb.tile([128, KT - h, cap], bf16, tag="xtb", bufs=1, name="xb")
                for k in range(KT):
                    dst = xa[:, k, :] if k < h else xb[:, k - h, :]
                    nc.gpsimd.dma_start(dst, xgd.ap()[:, k, :])
                xg_at = lambda k: xa[:, k, :] if k < h else xb[:, k - h, :]
            else:
                xg = sb.tile([128, KT, cap], bf16, tag="xbuf", bufs=2, name="xg")
                for k in range(KT):
                    nc.gpsimd.dma_start(xg[:, k, :], xgd.ap()[:, k, :])
                xg_at = lambda k: xg[:, k, :]
            cw = None
            if cwd is not None:
                cw = sb.tile([128, len(mtl)], f32, tag="cw", bufs=2, name="cw")
                nc.gpsimd.dma_start(cw[:], cwd.ap())
            # GEMM1 + silu*mul -> at (A^T, [i, tokens], bf16)
            at = sb.tile([128, it_cnt, cap], bf16, tag="at", bufs=2, name="at")
            for t in range(it_cnt):
                pair = []
                for par in (0, 1):
                    wgu = sb.tile([128, KT, 128], bf16, tag="wgu", bufs=4, name="wgu")
                    nc.sync.dma_start(wgu[:], wgud.ap()[2 * t + par])
                    row = []
                    for off, n in chunks:
                        p = ps.tile([128, n], f32, tag=f"ps{par}", bufs=3, name=f"ps{par}")
                        for k in range(KT):
                            nc.tensor.matmul(p[:], wgu[:, k, :], xg_at(k)[:, off:off + n],
                                             start=(k == 0), stop=(k == KT - 1))
                        row.append(p)
                    pair.append(row)
                for ci, (off, n) in enumerate(chunks):
                    tmp = sb.tile([128, n], bf16, tag="tmp", bufs=3, name="tmp")
                    if _SIM_SILU:
                        nc.scalar.activation(tmp[:], pair[0][ci][:], ACT_SIGMOID)
                        nc.vector.tensor_mul(tmp[:], tmp[:], pair[0][ci][:])
                    else:
                        nc.scalar.activation(tmp[:], pair[0][ci][:], ACT_SILU)
                    nc.vector.tensor_mul(at[:, t, off:off + n], tmp[:], pair[1][ci][:])
            # GEMM2 -> scale -> out
            for nt in range(NT2):
                wd = sb.tile([128, it_cnt, 512], bf16, tag="wd", bufs=3, name="wd")
                nc.sync.dma_start(wd[:], wdd.ap()[nt])
                for mi, (r0, p_) in enumerate(mtl):
                    yp = ps.tile([128, 512], f32, tag="psy", bufs=2, name="yp")
                    for k in range(it_cnt):
                        nc.tensor.matmul(yp[:p_, :], at[:, k, r0:r0 + p_], wd[:, k, :],
                                         start=(k == 0), stop=(k == it_cnt - 1))
                    ysb = sb.tile([128, 512], bf16, tag="ysb", bufs=3, name="ysb")
                    if cw is not None:
                        nc.vector.tensor_scalar_mul(ysb[:p_, :], yp[:p_, :], cw[:p_, mi:mi + 1])
                    else:
                        nc.vector.tensor_copy(ysb[:p_, :], yp[:p_, :])
                    nc.scalar.dma_start(out_d.ap()[r0:r0 + p_, nt * 512:(nt + 1) * 512],
                                        ysb[:p_, :])

        for s in range(NSLOT):
            ffn(xg_d[s], caps[s], _chunks(caps[s]), HIT, wgu_d[s], wd_d[s],
                cw_d[s], slot_mtl[s], yr_d[s])
        ffn(xt_d, T, [(0, 512), (512, 512)], SIT, wsgu_d, wsd_d, None, mts, ys_d)

    nc.compile()
    return nc


# ----------------------------------------------------------------- kernel
def kernel(x, gate_w, bias, w_gate_up, w_down, shared_w_gate_up,
           shared_w_down, _trace=False):
    x = np.ascontiguousarray(x, dtype=np.float32)
    topk_idx, w = _route(x, gate_w, bias)
    cw_full = w.astype(np.float32) * np.float32(ROUTED_SCALE)

    # expert -> token list + weight list
    toks, wts, counts = [], [], np.zeros(E, dtype=np.int64)
    for e in range(E):
        tsel, ksel = np.where(topk_idx == e)
        toks.append(tsel)
        wts.append(cw_full[tsel, ksel])
        counts[e] = len(tsel)

    # rank-sorted experts dealt into 4 slots x 4 groups; group p -> cores
    # (2p, 2p+1), each core computing one half of the intermediate dim.
    order = np.argsort(-counts, kind="stable")
    slot_experts = [[int(order[4 * s + p]) for p in range(4)] for s in range(NSLOT)]
    caps = [_pad16(int(max(counts[e] for e in slot_experts[s])))
            for s in range(NSLOT)]
    n_mt = [(caps[s] + 127) // 128 for s in range(NSLOT)]

    # pack per (group, slot, half) once; xg/cw shared by both cores of a pair
    xt_pack = _pack_xT(x, T)
    in_maps = []
    for c in range(NCORES):
        p, h = c // 2, c % 2
        m = {}
        for s in range(NSLOT):
            eid = slot_experts[s][p]
            if h == 0:
                m[f"xg{s}"] = _pack_xT(x[toks[eid]], caps[s])
                cwv = np.zeros(n_mt[s] * 128, dtype=np.float32)
                cwv[:counts[eid]] = wts[eid]
                m[f"cw{s}"] = np.ascontiguousarray(cwv.reshape(n_mt[s], 128).T)
            else:
                m[f"xg{s}"] = in_maps[c - 1][f"xg{s}"]
                m[f"cw{s}"] = in_maps[c - 1][f"cw{s}"]
            # half h of the expert's intermediate rows, zero-padded 704->768
            gsl = np.zeros((2 * IHP, D), dtype=np.float32)
            gsl[:IH] = w_gate_up[eid][IH * h: IH * (h + 1)]
            gsl[IHP:IHP + IH] = w_gate_up[eid][I + IH * h: I + IH * (h + 1)]
            m[f"wgu{s}"] = _pack_wgu(gsl, HIT)
            sdT = np.zeros((IHP, D), dtype=np.float32)
            sdT[:IH] = w_down[eid].T[IH * h: IH * (h + 1)]
            m[f"wd{s}"] = _pack_wd(sdT, HIT)
        # shared expert slice (rows [352c, 352c+352), zero-padded to 384)
        gsl = np.zeros((2 * SIP, D), dtype=np.float32)
        gsl[:SSLICE] = shared_w_gate_up[SSLICE * c: SSLICE * (c + 1)]
        gsl[SIP:SIP + SSLICE] = shared_w_gate_up[SI + SSLICE * c: SI + SSLICE * (c + 1)]
        m["wsgu"] = _pack_wgu(gsl, SIT)
        sdT = np.zeros((SIP, D), dtype=np.float32)
        sdT[:SSLICE] = shared_w_down[:, SSLICE * c: SSLICE * (c + 1)].T
        m["wsd"] = _pack_wd(sdT, SIT)
        m["xt"] = xt_pack
        in_maps.append(m)

    nc = _build(caps)
    kw = {}
    if _trace:
        kw = dict(trace=True, trace_cores=list(range(NCORES)))
    res = run_bass_kernel_spmd(nc, in_maps, core_ids=list(range(NCORES)), **kw)

    y = np.zeros((T, D), dtype=np.float32)
    for c in range(NCORES):
        y += res.results[c]["ys"].astype(np.float32)
    for c in range(NCORES):
        p = c // 2
        for s in range(NSLOT):
            eid = slot_experts[s][p]
            n = int(counts[eid])
            if n:
                y[toks[eid]] += res.results[c][f"yr{s}"][:n].astype(np.float32)
    if _trace:
        return y, res
    return y
